# revision 59
# baseline (speedup 1.0000x reference)
"""PRXAttention TRN2 kernel: 8-core SPMD (2 batches x 4 head-groups).

Per core (b, g): project q/k/v for 4 heads (img) + k/v (txt), RMSNorm via
PE ones-matmul partition reduction, RoPE with host-prepared pair-deinterleaved
tables (g_q/g_k folded in), softmax without max-subtraction (scores bounded),
masking via host-side zeroing of masked encoder tokens + Z correction,
partial out-projection for the 4 heads; host sums the 4 partials per batch.

Softmax Z per (lqc,h) is reduced with a DVE bf16 tree over the 20 exp
key-tiles (ACT runs exp only), then partition-reduced with two fp32
ones-matmuls; the out-projection is software-pipelined one query-chunk
behind attention so PE never waits on the Z chain. Cross-phase DMA
prefetch: phase-P weights load during phase T, next rep's phase-T
inputs load during phase A.
"""

import numpy as np
import ml_dtypes

bf16 = ml_dtypes.bfloat16

B, L_IMG, L_TXT = 2, 2048, 2048 // 4
D, H, DH = 2048, 16, 128
HPC = 4                      # heads per core
NCORES = 8
EPS = 1e-6
SM_SCALE = 1.0 / float(np.sqrt(DH))
NDT = D // 128               # 16 d-model tiles
NLC = L_IMG // 512           # 4 img l-chunks of 512
NKT_TXT = L_TXT // 128       # 4 txt key tiles
NKT = NKT_TXT + L_IMG // 128  # 20 key tiles of 128

_PROG = {}


def _build_program(reps=1):
    from contextlib import ExitStack

    import concourse.bacc as bacc
    import concourse.bass as bass
    import concourse.tile as tile
    from concourse import mybir

    f32 = mybir.dt.float32
    bf = mybir.dt.bfloat16
    f16 = mybir.dt.float16
    AF = mybir.ActivationFunctionType

    nc = bacc.Bacc("TRN2", target_bir_lowering=False)
    xt_d = nc.declare_dram_parameter("xt", [D, L_IMG], bf, isOutput=False)
    et_d = nc.declare_dram_parameter("et", [D, L_TXT], bf, isOutput=False)
    wq_d = nc.declare_dram_parameter("wq", [D, HPC * DH], bf, isOutput=False)
    wk_d = nc.declare_dram_parameter("wk", [D, HPC * DH], bf, isOutput=False)
    wv_d = nc.declare_dram_parameter("wv", [D, HPC * DH], bf, isOutput=False)
    wtk_d = nc.declare_dram_parameter("wtk", [D, HPC * DH], bf, isOutput=False)
    wtv_d = nc.declare_dram_parameter("wtv", [D, HPC * DH], bf, isOutput=False)
    wo_d = nc.declare_dram_parameter("wo", [HPC * DH, D], bf, isOutput=False)
    tq_d = nc.declare_dram_parameter("tq", [DH, 2, L_IMG], bf, isOutput=False)
    tk_d = nc.declare_dram_parameter("tk", [DH, 2, L_IMG], bf, isOutput=False)
    gtk_d = nc.declare_dram_parameter("gtk", [DH, 1], f32, isOutput=False)
    nm_d = nc.declare_dram_parameter("nm", [1, 1], f32, isOutput=False)
    out_d = nc.declare_dram_parameter("out", [L_IMG, D], bf, isOutput=True)

    with tile.TileContext(nc) as tc, ExitStack() as ctx:
        # ---- persistent pools (whole kernel) ----
        const = ctx.enter_context(tc.tile_pool(name="const", bufs=1))
        persist = ctx.enter_context(tc.tile_pool(name="persist", bufs=1))
        ps_s = ctx.enter_context(
            tc.tile_pool(name="ps_s", bufs=2, space=bass.MemorySpace.PSUM))
        ps_acc = ctx.enter_context(
            tc.tile_pool(name="ps_acc", bufs=2, space=bass.MemorySpace.PSUM))
        ps_op = ctx.enter_context(
            tc.tile_pool(name="ps_op", bufs=1, space=bass.MemorySpace.PSUM))
        ps_zb = ctx.enter_context(
            tc.tile_pool(name="ps_zb", bufs=1, space=bass.MemorySpace.PSUM))
        phOt = ctx.enter_context(tc.tile_pool(name="phOt", bufs=3))

        ones_col = const.tile([128, 1], bf, name="ones_col")
        nc.vector.memset(ones_col[:], 1.0)
        ones_row_h = const.tile([1, 128], f16, name="ones_row_h")
        nc.vector.memset(ones_row_h[:], 1.0)
        gtk_s = const.tile([DH, 1], f32, name="gtk_s")
        nc.sync.dma_start(gtk_s[:], gtk_d[:, :])
        nm_s = const.tile([1, 1], f32, name="nm_s")
        nc.sync.dma_start(nm_s[:], nm_d[:, :])
        eps_s = const.tile([1, 1], f32, name="eps_s")
        nc.vector.memset(eps_s[:], EPS)
        tqs = const.tile([DH, 2, L_IMG], bf, name="tqs")
        nc.sync.dma_start(tqs[:], tq_d[:, :, :])
        tks_tab = const.tile([DH, 2, L_IMG], bf, name="tks_tab")
        nc.sync.dma_start(tks_tab[:], tk_d[:, :, :])

        qf = [persist.tile([DH, L_IMG], bf, name=f"qf{h}", tag=f"qf{h}")
              for h in range(HPC)]
        kf = [persist.tile([DH, L_IMG], bf, name=f"kf{h}", tag=f"kf{h}")
              for h in range(HPC)]
        tkf = [persist.tile([DH, L_TXT], bf, name=f"tkf{h}", tag=f"tkf{h}")
               for h in range(HPC)]
        vs = persist.tile([128, NKT, HPC * DH], bf, name="vs")

        xt_r = xt_d[:, :].rearrange("(t p) l -> p t l", p=128)

        def rmsnorm_factor(pool_small, acc_psum):
            """acc_psum: [128, n] f32 projection output. Returns a [128, n]
            f32 PSUM broadcast of rsqrt(mean(x^2) + eps) per column."""
            n = acc_psum.shape[-1]
            sqt = pool_small.tile([128, n], bf, name="sqt", tag="sqt", bufs=2)
            nc.scalar.square(sqt[:], acc_psum)
            zp = ps_zb.tile([1, n], f32, name="zpt", tag="zb")[:, :]
            nc.tensor.matmul(zp, ones_col[:], sqt[:], start=True, stop=True)
            sq = pool_small.tile([1, n], f32, name="sq", tag="sq", bufs=1)
            nc.scalar.activation(sq[:], zp, AF.Sqrt, bias=eps_s[:],
                                 scale=1.0 / DH)
            rn = pool_small.tile([1, n], f32, name="rn", tag="rn", bufs=1)
            nc.vector.reciprocal(rn[:], sq[:])
            rnh = pool_small.tile([1, n], f16, name="rnh", tag="rnh", bufs=1)
            nc.scalar.copy(rnh[:], rn[:])
            nb = ps_zb.tile([128, n], f32, name="nbt", tag="zb")[:, :]
            nc.tensor.matmul(nb, ones_row_h[:], rnh[:], start=True, stop=True)
            return nb

        # phase-T inputs + phase-P q/k weights: persistent tiles, re-DMA'd
        # per rep (the DMAs for rep r+1 are emitted during rep r's phase A
        # so they prefetch behind the prior rep's compute).
        pT = ctx.enter_context(tc.tile_pool(name="pT", bufs=1))

        def emit_phT_dmas():
            ets = pT.tile([128, NDT, L_TXT], bf, name="ets", tag="ets")
            nc.sync.dma_start(
                ets[:], et_d[:, :].rearrange("(t p) l -> p t l", p=128))
            wtks = pT.tile([128, NDT, HPC * DH], bf, name="wtks", tag="wtks")
            nc.sync.dma_start(
                wtks[:], wtk_d[:, :].rearrange("(t p) m -> p t m", p=128))
            wqs = pT.tile([128, NDT, HPC * DH], bf, name="wqs", tag="wqs")
            nc.sync.dma_start(
                wqs[:], wq_d[:, :].rearrange("(t p) m -> p t m", p=128))
            wks = pT.tile([128, NDT, HPC * DH], bf, name="wks", tag="wks")
            nc.sync.dma_start(
                wks[:], wk_d[:, :].rearrange("(t p) m -> p t m", p=128))
            return ets, wtks, wqs, wks

        tilesT = emit_phT_dmas()
        for r in range(reps):
            ets, wtks, wqs, wks = tilesT
            stP = ExitStack()
            phPx = stP.enter_context(tc.tile_pool(name="phPx", bufs=2))
            xs0 = phPx.tile([128, NDT, 512], bf, name="xs", tag="xs")
            nc.sync.dma_start(xs0[:], xt_r[:, :, 0:512])

            # ================= phase T: text k/v =================
            with tc.tile_pool(name="phTt", bufs=2) as phTt:
                wtvs = phTt.tile([128, NDT, HPC * DH], bf, name="wtvs",
                                 tag="wtvs", bufs=1)
                nc.sync.dma_start(
                    wtvs[:], wtv_d[:, :].rearrange("(t p) m -> p t m", p=128))
                def finish_k(h, kp):
                    ksc = phTt.tile([128, L_TXT], bf, name="ksc", tag="ksc")
                    nc.scalar.activation(ksc[:], kp[:], AF.Copy, scale=gtk_s[:])
                    nb = rmsnorm_factor(phTt, kp[:])
                    nc.vector.tensor_mul(tkf[h][:, :], ksc[:], nb[:])

                # emit each head's rmsnorm chain one K-proj group behind, so
                # PE never waits on the ACT-side chain at the cold rep start
                kps = []
                for h in range(HPC):
                    kp = ps_acc.tile([128, L_TXT], f32, name="kp", tag="acc")
                    for d in range(NDT):
                        nc.tensor.matmul(kp[:], wtks[:, d, h * DH:(h + 1) * DH],
                                         ets[:, d, :],
                                         start=(d == 0), stop=(d == NDT - 1))
                    kps.append(kp)
                    if h >= 1:
                        finish_k(h - 1, kps[h - 1])
                for lt in range(NKT_TXT):
                    vp = ps_acc.tile([128, HPC * DH], f32, name="vp", tag="acc")
                    for d in range(NDT):
                        nc.tensor.matmul(vp[:],
                                         ets[:, d, lt * 128:(lt + 1) * 128],
                                         wtvs[:, d, :],
                                         start=(d == 0), stop=(d == NDT - 1))
                    if lt == 0:
                        finish_k(HPC - 1, kps[HPC - 1])
                    nc.scalar.copy(vs[:, lt, :], vp[:])

            # ============== phase P: image q/k/v projections ==============
            phPv = stP.enter_context(tc.tile_pool(name="phPv", bufs=1))
            wvs = phPv.tile([128, NDT, HPC * DH], bf, name="wvs")
            nc.sync.dma_start(
                wvs[:], wv_d[:, :].rearrange("(t p) m -> p t m", p=128))
            with tc.tile_pool(name="phPt", bufs=2) as phPt:
                for lc in range(NLC):
                    lsl = slice(lc * 512, (lc + 1) * 512)
                    if lc == 0:
                        xs = xs0
                    else:
                        xs = phPx.tile([128, NDT, 512], bf, name="xs", tag="xs")
                        nc.sync.dma_start(xs[:], xt_r[:, :, lsl])
                    for h in range(HPC):
                        for wt, tab, dst in ((wqs, tqs, qf[h]),
                                             (wks, tks_tab, kf[h])):
                            pp = ps_acc.tile([128, 512], f32, name="pp",
                                             tag="acc")
                            for d in range(NDT):
                                nc.tensor.matmul(
                                    pp[:], wt[:, d, h * DH:(h + 1) * DH],
                                    xs[:, d, :],
                                    start=(d == 0), stop=(d == NDT - 1))
                            ev = phPt.tile([128, 512], bf, name="ev", tag="ev")
                            nc.scalar.copy(ev[:], pp[:])
                            nb = rmsnorm_factor(phPt, pp[:])
                            # rope then norm: dst = (tabA*ev + tabB*swap64(ev))*nb
                            evsA = phPt.tile([128, 512], bf, name="evsA",
                                             tag="evsA", bufs=1)
                            nc.sync.dma_start(evsA[0:64, :], ev[64:128, :])
                            evsB = phPt.tile([128, 512], bf, name="evsB",
                                             tag="evsB", bufs=1)
                            nc.sync.dma_start(evsB[64:128, :], ev[0:64, :])
                            rA = phPt.tile([128, 512], bf, name="rA", tag="rA")
                            nc.vector.tensor_mul(rA[:], ev[:], tab[:, 0, lsl])
                            rB = phPt.tile([128, 512], bf, name="rB", tag="rB")
                            nc.vector.tensor_mul(rB[0:64, :], evsA[0:64, :],
                                                 tab[0:64, 1, lsl])
                            nc.vector.tensor_mul(rB[64:128, :], evsB[64:128, :],
                                                 tab[64:128, 1, lsl])
                            rs = phPt.tile([128, 512], bf, name="rs", tag="rs")
                            nc.vector.tensor_add(rs[:], rA[:], rB[:])
                            nc.vector.tensor_mul(dst[:, lsl], rs[:], nb[:])
                    for ltl in range(4):
                        vp = ps_acc.tile([128, HPC * DH], f32, name="vpi",
                                         tag="acc")
                        for d in range(NDT):
                            nc.tensor.matmul(
                                vp[:], xs[:, d, ltl * 128:(ltl + 1) * 128],
                                wvs[:, d, :], start=(d == 0),
                                stop=(d == NDT - 1))
                        nc.scalar.copy(vs[:, NKT_TXT + lc * 4 + ltl, :], vp[:])
            stP.close()

            # ====== phase A+O: attention with pipelined out-projection ======
            with tc.tile_pool(name="phA", bufs=1) as phA, \
                 tc.tile_pool(name="phAt", bufs=2) as phAt, \
                 tc.tile_pool(name="phAtr", bufs=1) as phAtr, \
                 tc.tile_pool(name="phAv", bufs=2) as phAv:
                pt = phA.tile([128, NKT * 512], bf, name="pt")
                wos = [phA.tile([DH, D], bf, name=f"wos{h}", tag=f"wos{h}")
                       for h in range(HPC)]
                for h in range(HPC):
                    nc.sync.dma_start(wos[h][:], wo_d[h * DH:(h + 1) * DH, :])
                # prefetch next rep's phase-T inputs during phase A
                if r + 1 < reps:
                    tilesT = emit_phT_dmas()

                afv_tiles = {}

                def attention(lqc, opgen=None):
                    qsl = slice(lqc * 512, (lqc + 1) * 512)
                    afv = phAv.tile([128, HPC, 512], bf, name="afv", tag="afv")
                    afv_tiles[lqc] = afv
                    for h in range(HPC):
                        av = ps_acc.tile([128, 512], f32, name="av", tag="acc")
                        t5 = None
                        for lkp in range(NKT // 2 + 1):
                            if opgen is not None and 2 <= lkp <= 9:
                                next(opgen, None)
                            if lkp < NKT // 2:
                                spw = ps_s.tile([128, 1024], f32, name="spw",
                                                tag="s")
                                for half in range(2):
                                    lk = 2 * lkp + half
                                    if lk < NKT_TXT:
                                        lhsT = tkf[h][:, lk * 128:
                                                      (lk + 1) * 128]
                                    else:
                                        lhsT = kf[h][:, (lk - NKT_TXT) * 128:
                                                     (lk - NKT_TXT + 1) * 128]
                                    nc.tensor.matmul(
                                        spw[:, half * 512:(half + 1) * 512],
                                        lhsT, qf[h][:, qsl],
                                        start=True, stop=True)
                                nc.scalar.activation(
                                    pt[:, lkp * 1024:(lkp + 1) * 1024],
                                    spw[:], AF.Exp, scale=SM_SCALE)
                            if lkp == 6:
                                # tiles 0..9 exp'd; start the Z tree early
                                # (on the otherwise-idle Pool engine)
                                t5 = phAtr.tile([128, 5 * 512], bf, name="t5",
                                                tag="t5")
                                nc.vector.tensor_add(t5[:], pt[:, 0:2560],
                                                     pt[:, 2560:5120])
                            if lkp == 9:
                                # tiles 10..14 exp'd (pair 7 done at lkp 8)
                                nc.vector.tensor_add(t5[:], t5[:],
                                                     pt[:, 5120:7680])
                                # tiles 15..17 too (pair 8 done)
                                nc.vector.tensor_add(t5[:, 0:1536],
                                                     t5[:, 0:1536],
                                                     pt[:, 7680:9216])
                            jp = lkp - 1
                            if jp >= 0:
                                for half in range(2):
                                    j = 2 * jp + half
                                    pj = pt[:, j * 512:(j + 1) * 512]
                                    nc.tensor.matmul(
                                        av[:], vs[:, j, h * DH:(h + 1) * DH],
                                        pj, start=(j == 0),
                                        stop=(j == NKT - 1))
                        # finish Z tree: t5 += tiles 18-19, then fold the 5
                        # partials down in place
                        nc.vector.tensor_add(t5[:, 1536:2560],
                                             t5[:, 1536:2560],
                                             pt[:, 9216:10240])
                        nc.vector.tensor_add(t5[:, 0:1024], t5[:, 0:1024],
                                             t5[:, 1024:2048])
                        nc.vector.tensor_add(t5[:, 0:512], t5[:, 0:512],
                                             t5[:, 512:1024])
                        paS = phAt.tile([128, 512], bf, name="paS", tag="paS")
                        nc.vector.tensor_add(paS[:], t5[:, 0:512],
                                             t5[:, 2048:2560])
                        zp = ps_zb.tile([1, 512], f32, name="zpa",
                                        tag="zb")[:, :]
                        nc.tensor.matmul(zp, ones_col[:], paS[:],
                                         start=True, stop=True)
                        zs = phAt.tile([1, 512], f32, name="zs", tag="zs",
                                       bufs=1)
                        nc.vector.tensor_scalar_add(zs[:], zp, nm_s[:])
                        rz = phAt.tile([1, 512], f32, name="rz", tag="rz",
                                       bufs=1)
                        nc.vector.reciprocal(rz[:], zs[:])
                        rzh = phAt.tile([1, 512], f16, name="rzh", tag="rzh",
                                        bufs=1)
                        nc.vector.tensor_copy(rzh[:], rz[:])
                        nb2 = ps_zb.tile([128, 512], f32, name="nb2",
                                         tag="zb")[:, :]
                        nc.tensor.matmul(nb2, ones_row_h[:], rzh[:],
                                         start=True, stop=True)
                        avs = phAt.tile([128, 512], bf, name="avs", tag="avs")
                        nc.vector.tensor_copy(avs[:], av[:])
                        nc.vector.tensor_mul(afv[:, h, :], avs[:], nb2)

                def outproj(lqc, last=False):
                    """Generator: one yield per out-proj matmul group, so the
                    groups can interleave into the next attention's PE stream."""
                    afv = afv_tiles.pop(lqc)
                    gi = 0
                    for ltl in range(4):
                        row0 = lqc * 512 + ltl * 128
                        for dc in range(D // 512):
                            if last and gi % 4 < 2:
                                # attention is done: rotate through the idle
                                # ps_s banks too, so the copies never pace PE
                                op = ps_s.tile([128, 1024], f32, name="op",
                                               tag="s")[:, 0:512]
                            elif last and gi % 4 == 3:
                                op = ps_zb.tile([128, 512], f32, name="op",
                                                tag="zb")[:, :]
                            else:
                                op = ps_op.tile([128, 512], f32, name="op",
                                                tag="op")[:, :]
                            gi += 1
                            for hh in range(HPC):
                                nc.tensor.matmul(
                                    op[:],
                                    afv[:, hh, ltl * 128:(ltl + 1) * 128],
                                    wos[hh][:, dc * 512:(dc + 1) * 512],
                                    start=(hh == 0), stop=(hh == HPC - 1))
                                if hh == 1 and not last:
                                    yield
                            os_t = phOt.tile([128, 512], bf, name="os",
                                             tag="os")
                            if last and dc % 2 == 1:
                                # rep-end: DVE is draining the Z tail; ACT
                                # is idle — split the copies between them
                                nc.scalar.copy(os_t[:], op[:])
                            else:
                                nc.vector.tensor_copy(os_t[:], op[:])
                            nc.sync.dma_start(
                                out_d[row0:row0 + 128,
                                      dc * 512:(dc + 1) * 512], os_t[:])
                            yield

                for lqc in range(NLC):
                    gen = outproj(lqc - 1) if lqc > 0 else None
                    attention(lqc, opgen=gen)
                    if gen is not None:
                        for _ in gen:
                            pass
                for _ in outproj(NLC - 1, last=True):
                    pass

    nc.finalize()
    return nc


def _get_program(reps=1):
    if reps not in _PROG:
        _PROG[reps] = _build_program(reps=reps)
    return _PROG[reps]


_PERM = np.concatenate([np.arange(0, DH, 2), np.arange(1, DH, 2)])


def make_core_inputs(inputs: dict) -> list:
    hs = np.asarray(inputs["hidden_states"], np.float32)
    enc = np.asarray(inputs["encoder_hidden_states"], np.float32)
    mask = np.asarray(inputs["attention_mask"]).astype(bool)
    emb = np.asarray(inputs["image_rotary_emb"], np.float32)
    wqkv = np.asarray(inputs["w_img_qkv"], np.float32).reshape(D, 3, H, DH)
    wtkv = np.asarray(inputs["w_txt_kv"], np.float32).reshape(D, 2, H, DH)
    wout = np.asarray(inputs["w_out"], np.float32).reshape(H, DH, D)
    g_q = np.asarray(inputs["g_q"], np.float32)
    g_k = np.asarray(inputs["g_k"], np.float32)
    g_ak = np.asarray(inputs["g_added_k"], np.float32)

    def tables(F, g):
        # F: [L, 64, 2, 2]; permuted layout: part p<64 -> dim 2p, 64+p -> 2p+1
        # dst = tabA * ev + tabB * swap64(ev)
        ge, go = g[0::2], g[1::2]
        tabA = np.concatenate([(F[:, :, 0, 0] * ge[None, :]).T,
                               (F[:, :, 1, 1] * go[None, :]).T], axis=0)
        tabB = np.concatenate([(F[:, :, 0, 1] * go[None, :]).T,
                               (F[:, :, 1, 0] * ge[None, :]).T], axis=0)
        return np.stack([tabA, tabB], axis=1).astype(bf16)  # [128, 2, L]

    in_maps = []
    for c in range(NCORES):
        b, g = divmod(c, 4)
        hsel = slice(g * HPC, (g + 1) * HPC)
        F = emb[b, 0]
        wq = wqkv[:, 0, hsel, :][:, :, _PERM].reshape(D, HPC * DH)
        wk = wqkv[:, 1, hsel, :][:, :, _PERM].reshape(D, HPC * DH)
        wv = wqkv[:, 2, hsel, :].reshape(D, HPC * DH)
        wtk = wtkv[:, 0, hsel, :][:, :, _PERM].reshape(D, HPC * DH)
        wtv = wtkv[:, 1, hsel, :].reshape(D, HPC * DH)
        wo = wout[hsel].reshape(HPC * DH, D)
        in_maps.append({
            "xt": np.ascontiguousarray(hs[b].T).astype(bf16),
            "et": np.ascontiguousarray((enc[b] * mask[b][:, None]).T).astype(bf16),
            "wq": np.ascontiguousarray(wq).astype(bf16),
            "wk": np.ascontiguousarray(wk).astype(bf16),
            "wv": np.ascontiguousarray(wv).astype(bf16),
            "wtk": np.ascontiguousarray(wtk).astype(bf16),
            "wtv": np.ascontiguousarray(wtv).astype(bf16),
            "wo": np.ascontiguousarray(wo).astype(bf16),
            "tq": tables(F, g_q),
            "tk": tables(F, g_k),
            "gtk": g_ak[_PERM].reshape(DH, 1).astype(np.float32),
            "nm": np.array([[-(float(L_TXT) - float(mask[b].sum()))]], np.float32),
        })
    return in_maps


def run_cores(in_maps, trace=False, tmpdir=None):
    from concourse.bass_utils import run_bass_kernel_spmd
    nc = _get_program()
    return run_bass_kernel_spmd(nc, in_maps, list(range(NCORES)),
                                trace=trace, tmpdir=tmpdir)


def make_runner(reps=1, ncores=NCORES):
    import jax
    from jax.sharding import Mesh, PartitionSpec
    from jax.experimental.shard_map import shard_map

    from concourse import bass2jax, mybir

    nc = _get_program(reps)
    bass2jax.install_neuronx_cc_hook()

    partition_name = (nc.partition_id_tensor.name
                      if nc.partition_id_tensor else None)
    in_names, out_names, out_avals, zero_outs = [], [], [], []
    for alloc in nc.m.functions[0].allocations:
        if not isinstance(alloc, mybir.MemoryLocationSet):
            continue
        name = alloc.memorylocations[0].name
        if alloc.kind == "ExternalInput":
            if name != partition_name:
                in_names.append(name)
        elif alloc.kind == "ExternalOutput":
            out_names.append(name)
            shape = tuple(alloc.tensor_shape)
            dtype = mybir.dt.np(alloc.dtype)
            out_avals.append(jax.core.ShapedArray(shape, dtype))
            zero_outs.append(np.zeros(shape, dtype))
    n_params = len(in_names)
    all_names = in_names + out_names
    if partition_name is not None:
        all_names.append(partition_name)

    def _body(*args):
        operands = list(args)
        if partition_name is not None:
            operands.append(bass2jax.partition_id_tensor())
        return tuple(bass2jax._bass_exec_p.bind(
            *operands,
            out_avals=tuple(out_avals),
            in_names=tuple(all_names),
            out_names=tuple(out_names),
            lowering_input_output_aliases=(),
            sim_require_finite=True,
            sim_require_nnan=True,
            nc=nc,
        ))

    devices = jax.devices()[:ncores]
    mesh = Mesh(np.asarray(devices), ("core",))
    nin = n_params + len(out_names)
    sharded = jax.jit(shard_map(
        _body, mesh=mesh,
        in_specs=(PartitionSpec("core"),) * nin,
        out_specs=(PartitionSpec("core"),) * len(out_names),
        check_rep=False))

    def prep_args(in_maps):
        concat_in = [np.concatenate([in_maps[c][nm] for c in range(ncores)],
                                    axis=0) for nm in in_names]
        concat_zero = [np.zeros((ncores * z.shape[0], *z.shape[1:]), z.dtype)
                       for z in zero_outs]
        sh = jax.sharding.NamedSharding(mesh, PartitionSpec("core"))
        return [jax.device_put(a, sh) for a in (*concat_in, *concat_zero)]

    return sharded, prep_args


def time_cores(in_maps, iters=30, reps=1, ncores=NCORES):
    import time
    import jax

    sharded, prep_args = make_runner(reps, ncores)
    dev_args = prep_args(in_maps)
    jax.block_until_ready(sharded(*dev_args))
    times = []
    for _ in range(iters):
        t0 = time.perf_counter()
        out = sharded(*dev_args)
        jax.block_until_ready(out)
        times.append(time.perf_counter() - t0)
    times_ns = sorted(int(t * 1e9) for t in times)
    return times_ns


def bench_paired(in_maps, rounds=40, r_hi=4, ncores=1):
    """Median per-rep HW time from interleaved reps=1 / reps=r_hi rounds.

    Each round runs [T1, T_hi, T_hi, T1]; the within-round differential
    (sum(T_hi) - sum(T1)) / (2*(r_hi-1)) cancels the slow drift of the
    axon dispatch floor that corrupts min-of-separate-sets differentials.
    r_hi matches the burst length of the original (T4-T1)/3 methodology:
    longer bursts inflate per-rep time via the sustained-power downclock.
    """
    import time
    import jax

    s1, prep1 = make_runner(1, ncores)
    s4, prep4 = make_runner(r_hi, ncores)
    a1 = prep1(in_maps)
    a4 = prep4(in_maps)
    jax.block_until_ready(s1(*a1))
    jax.block_until_ready(s4(*a4))
    diffs = []
    for _ in range(rounds):
        t0 = time.perf_counter(); jax.block_until_ready(s1(*a1))
        t1 = time.perf_counter(); jax.block_until_ready(s4(*a4))
        t2 = time.perf_counter(); jax.block_until_ready(s4(*a4))
        t3 = time.perf_counter(); jax.block_until_ready(s1(*a1))
        t4 = time.perf_counter()
        diffs.append((((t2 - t1) + (t3 - t2)) - ((t1 - t0) + (t4 - t3)))
                     / (2 * (r_hi - 1)) * 1e9)
    diffs = sorted(diffs)
    return int(diffs[len(diffs) // 2])


def kernel(**inputs) -> np.ndarray:
    in_maps = make_core_inputs(inputs)
    res = run_cores(in_maps)
    out = np.zeros((B, L_IMG, D), np.float32)
    for c in range(NCORES):
        b = c // 4
        out[b] += np.asarray(res.results[c]["out"], np.float32)
    return out
